# revision 61
# baseline (speedup 1.0000x reference)
"""Boid policy kernel for Trainium2 (8 NeuronCores).

Strategy
--------
Host: sort boids into 16 spatial patches (4 x-strips x 4 y-quantiles of
exactly 512 queries). Each core processes 2 patches; coordinates per patch
are shifted (mod 1, bit-exact on the 2^-23 lattice) so unwrapped diffs
equal toroidal diffs for every pair that can interact. Per patch the host
gathers candidates within perception reach of the patch rectangle
(~26 blocks of 128), sep-reach candidates in the leading (even-rounded)
NS blocks.

Device (per patch, j = candidate on partitions, i = query on free axis):
  * sep-reach blocks use the exact path (separation borderline pairs need
    bit-identical f32 arithmetic vs the reference -- ~20 pairs sit within
    1e-7 of the threshold):
      dx2 = ACT Square(qx_bcast + (-cx_j)), dy2 likewise (fp16 qx input
      upconverts exactly), d2 = dx2 + dy2 on GPSIMD (SBUF, bit-exact).
  * perception-only blocks compute d2 on the PE as a K=10-feature bf16
    matmul into PSUM, two blocks per 2-bank PSUM pair tile (bf16 2-way
    lattice splits; |err| ~ 1e-5, harmless at the 0.04 threshold).
  * masks: one elementwise op per PAIR of blocks ([128, 2, C] tiles),
    split DVE/ACT with per-engine conventions fixed up on the host:
      DVE : (d2<=T) - 0.5 -> +-0.5   (full weights,  corr 0.5*sum w)
      ACT : Sign(T - d2)   -> +-1    (half weights,  corr 0.5*sum 2w)
    Perception masks for all pairs; separation masks additionally for the
    sep pairs (from the exact d2). Masks are written in fp8e4.
  * aggregation as fp8e4 DoubleRow matmuls (one matmul contracts TWO
    blocks): perception 11 features [cnt, vx*2, vy*2, px*3, py*3],
    separation 9 [cnt, px*4, py*4], weights as 16^-i-scaled e4m3 splits.
  * PE work is emitted in groups of 4 d2p columns-matmuls; agg flushes are
    batched every other group boundary (amortizes PE dtype-mode switches);
    warmup matmuls into the (later zeroed) slot-1 accumulators ramp the PE
    during the input DMA.

Host epilogue (f64): add mask-convention corrections, recombine the split
features, recover sum(mask*diff) = sum(mask*pc) - qc*count, subtract
self, zero the separation steer for boids with no sep neighbors (exact
count), normalize the three steers, combine, add noise, clip.
"""

import numpy as np
import ml_dtypes

import concourse.bass as bass
import concourse.bacc as bacc
import concourse.mybir as mybir
from concourse.tile import TileContext
from concourse.bass_utils import run_bass_kernel_spmd

N = 8192
NCORES = 8
NPATCH = 16
C = N // NPATCH  # 512 queries per patch
PERC2 = float(np.float32(0.2**2))
SEP2 = float(np.float32(0.02**2))
EPS = 1e-8
RCULL_P = 0.2 + 1e-3
RCULL_S = 0.02 + 1e-3
SCL = 1 << 23
E4M3 = ml_dtypes.float8_e4m3
BF16 = ml_dtypes.bfloat16
G = 4          # d2p group size in blocks (= 2 PSUM pair tiles)
NWARM = 3      # warmup matmuls during input DMA
MP = 11        # perception agg features
MS = 9         # separation agg features

_CACHE = {}


def _pair_order(NP2, NS2):
    """Processing order of pairs: 2 perc pairs first (PE ramp), 3 perc
    pairs last (so ACT square bursts never clog the drain), sep pairs
    (exact ACT path, no PE work) spread evenly over the middle."""
    perc = list(range(NS2, NP2))
    sep = list(range(NS2))
    nh, nt = 2, 3
    head = perc[:nh]
    tail = perc[max(len(perc) - nt, nh):]
    rest = perc[nh:max(len(perc) - nt, nh)]
    res = []
    n = max(len(rest), 1)
    m = len(sep)
    si = 0
    for i, p in enumerate(rest):
        res.append(p)
        while si < m and (i + 1) * m >= (si + 1) * n:
            res.append(sep[si])
            si += 1
    res.extend(sep[si:])
    return head + res + tail


def _mask_jobs(NP, NSe):
    """Pair-level mask jobs in processing order + greedy min-finish-time
    engine assignment (keyed by block for the host's weight packing; both
    blocks of a pair share one mask op and engine). Simulates each
    engine's clock: a sep pair adds its 4 ACT squares (~0.80us each) to
    ACT's clock before its masks are placed.
    """
    NP2 = (NP + 1) // 2
    NS2 = NSe // 2
    cost = {"v": 1.10, "a": 0.85}
    sq = 0.80
    used = {"v": 0.0, "a": 0.0}
    out = []

    def assign(kind, k, engines):
        e = min(engines, key=lambda e: used[e] + cost[e])
        used[e] += cost[e]
        for b in (2 * k, 2 * k + 1):
            out.append((kind, b, e))

    order = _pair_order(NP2, NS2)
    for i, k in enumerate(order):
        if k < NS2:
            used["a"] += 4 * sq    # ACT squares for this pair
            assign("p", k, ("v", "a"))
            # NOTE: Pool tensor_scalar is a ~16us ucode path - never
            # assign masks to it
            assign("s", k, ("v", "a"))
        elif i < 2:
            # head pairs: per-lane masks on both engines halve the first
            # masks' latency, unblocking the dp-buffer rotation sooner
            for b in (2 * k, 2 * k + 1):
                e = min(("v", "a"), key=lambda e: used[e] + cost[e] / 2)
                used[e] += cost[e] / 2
                out.append(("p", b, e))
        else:
            assign("p", k, ("v", "a"))
    return out


def _build(cfg):
    """cfg = ((NP0, NSe0), (NP1, NSe1)); NSe even, NP >= NSe."""
    f32 = mybir.dt.float32
    f16 = mybir.dt.float16
    bf16 = mybir.dt.bfloat16
    f8 = mybir.dt.float8e4
    AF = mybir.ActivationFunctionType
    ALU = mybir.AluOpType
    DR = mybir.MatmulPerfMode.DoubleRow

    nc = bacc.Bacc()
    aux_w = [2 + 2 * NSe for _, NSe in cfg]
    aux_h = nc.declare_dram_parameter(
        "aux", [128, sum(aux_w)], f32, isOutput=False)
    ffc_h = []
    jw_h = []
    qxy_h = []
    for s, (NP, NSe) in enumerate(cfg):
        NPP = NP - NSe            # perc-only blocks (d2p path)
        NPe = NP + (NP & 1)
        ffc_h.append(nc.declare_dram_parameter(
            f"ffc{s}", [10, C + 128 * NPP], bf16, isOutput=False))
        jw_h.append(nc.declare_dram_parameter(
            f"jw{s}", [128, 32 * (NPe // 2) + 32 * (NSe // 2)], f8,
            isOutput=False))
        qxy_h.append(nc.declare_dram_parameter(
            f"qxy{s}", [128, 2 * C], f32, isOutput=False))
    outp_h = nc.declare_dram_parameter("outp", [MP, 2 * C], f32, isOutput=True)
    outs_h = nc.declare_dram_parameter("outs", [MS, 2 * C], f32, isOutput=True)

    with TileContext(nc) as tc:
        with (
            tc.tile_pool(name="const", bufs=1) as cpool,
            tc.tile_pool(name="work", bufs=4) as wpool,
            tc.tile_pool(name="acc", bufs=1, space="PSUM") as apool,
        ):
            aux = cpool.tile([128, sum(aux_w)], f32)
            nc.gpsimd.dma_start(out=aux[:], in_=aux_h[:, :])
            tiles = []
            for s, (NP, NSe) in enumerate(cfg):
                NPP = NP - NSe
                NPe = NP + (NP & 1)
                t = {"NP": NP, "NSe": NSe, "NPP": NPP, "NPe": NPe}
                t["fq"] = cpool.tile([10, C], bf16, name=f"fq{s}")
                nc.sync.dma_start(out=t["fq"][:], in_=ffc_h[s][:, 0:C])
                B1 = min(NPP, 2 * G)
                fca = cpool.tile([10, 128 * max(B1, 1)], bf16, name=f"fca{s}")
                if B1 > 0:
                    nc.sync.dma_start(out=fca[:, 0:128 * B1],
                                      in_=ffc_h[s][:, C:C + 128 * B1])
                fcb = None
                if NPP > B1:
                    fcb = cpool.tile([10, 128 * (NPP - B1)], bf16,
                                     name=f"fcb{s}")
                    nc.sync.dma_start(out=fcb[:],
                                      in_=ffc_h[s][:, C + 128 * B1:])
                t["fc"] = lambda j, fca=fca, fcb=fcb, B1=B1: (
                    fca[:, 128 * j:128 * (j + 1)] if j < B1
                    else fcb[:, 128 * (j - B1):128 * (j - B1 + 1)])
                qxy = cpool.tile([128, 2 * C], f32, name=f"qxy{s}")
                nc.gpsimd.dma_start(out=qxy[:], in_=qxy_h[s][:, :])
                t["qx"] = qxy[:, 0:C]
                t["qy"] = qxy[:, C:2 * C]
                a0 = sum(aux_w[:s])
                t["thr"] = aux[:, a0:a0 + 1]       # PERC2
                t["thrs"] = aux[:, a0 + 1:a0 + 2]  # SEP2
                t["jbs"] = aux[:, a0 + 2:a0 + 2 + 2 * NSe]
                jw = cpool.tile([128, NPe // 2 + NSe // 2, 2, 16], f8,
                                name=f"jw{s}")
                nc.gpsimd.dma_start(out=jw[:], in_=jw_h[s][:, :])
                t["jwp"] = lambda k, jw=jw: jw[:, k, 0:2, 0:MP]
                t["jws"] = lambda k, jw=jw, NPe=NPe: \
                    jw[:, NPe // 2 + k, 0:2, 0:MS]
                t["accp"] = apool.tile([MP, C], f32, name=f"accp{s}")
                t["accs"] = apool.tile([MS, C], f32, name=f"accs{s}")
                t["jobs"] = _mask_jobs(NP, NSe)
                tiles.append(t)

            # warmup: ramp the PE while input DMAs land (results junk,
            # zeroed later by slot-1's first real start=True aggs)
            for w in range(NWARM):
                tgt = tiles[1]["accp" if w % 2 == 0 else "accs"]
                mw = MP if w % 2 == 0 else MS
                nc.tensor.matmul(out=tgt[:], lhsT=tiles[0]["fq"][:, 0:mw],
                                 rhs=tiles[0]["fq"][:], start=True, stop=True)

            # unified pipeline over both slots; "groups" of 2 pairs:
            # sep pairs first (exact path, no PE), then perc-only pairs
            sched = []  # (slot, [pair indices])
            for s, t in enumerate(tiles):
                NP2 = (t["NP"] + 1) // 2
                NS2 = t["NSe"] // 2
                prs = _pair_order(NP2, NS2)
                for g in range(0, len(prs), 2):
                    sched.append((s, prs[g:g + 2]))
            pend = []   # (boundary_done, slot, kind, pair_idx)
            pcnt = [0, 0]
            scnt = [0, 0]
            ppair = [{}, {}]
            spair = [{}, {}]

            def flush(job):
                bdone, s, kind, k = job[:4]
                t = tiles[s]
                if kind == "p":
                    tot = t["NPe"] // 2
                    nc.tensor.matmul(
                        out=t["accp"][:], lhsT=t["jwp"](k),
                        rhs=ppair[s][k][:, :, :],
                        start=(pcnt[s] == 0), stop=(pcnt[s] == tot - 1),
                        perf_mode=DR)
                    pcnt[s] += 1
                    if pcnt[s] == tot:
                        po = wpool.tile([MP, C], f32, tag=f"po{s}", bufs=1)
                        nc.scalar.copy(out=po[:], in_=t["accp"][:])
                        nc.sync.dma_start(
                            out=outp_h[:, C * s:C * (s + 1)], in_=po[:])
                else:
                    tot = t["NSe"] // 2
                    nc.tensor.matmul(
                        out=t["accs"][:], lhsT=t["jws"](k),
                        rhs=spair[s][k][:, :, :],
                        start=(scnt[s] == 0), stop=(scnt[s] == tot - 1),
                        perf_mode=DR)
                    scnt[s] += 1
                    if scnt[s] == tot:
                        so = wpool.tile([MS, C], f32, tag=f"so{s}", bufs=1)
                        nc.vector.tensor_copy(out=so[:], in_=t["accs"][:])
                        nc.sync.dma_start(
                            out=outs_h[:, C * s:C * (s + 1)], in_=so[:])

            def emit_mask(eng, out_ap, in_ap, thr_col, thr_f):
                if eng == "a":
                    nc.scalar.activation(out=out_ap, in_=in_ap,
                                         func=AF.Sign, bias=thr_col,
                                         scale=-1.0)
                elif eng == "g":
                    nc.gpsimd.tensor_scalar(out=out_ap, in0=in_ap,
                                            scalar1=thr_f, scalar2=None,
                                            op0=ALU.is_le)
                else:
                    nc.vector.tensor_scalar(out=out_ap, in0=in_ap,
                                            scalar1=thr_f, scalar2=0.5,
                                            op0=ALU.is_le, op1=ALU.subtract)

            nbound = len(sched) + 2
            for gi in range(nbound):
                s, grp = sched[gi] if gi < len(sched) else (None, [])
                dps = {}
                if grp:
                    t = tiles[s]
                    NS2 = t["NSe"] // 2
                    eng = {(kind, k): e for kind, k, e in t["jobs"]}
                    # PE d2p for perc-only pairs of this group
                    for k in grp:
                        if k < NS2:
                            continue
                        dp = apool.tile([128, 2, C], f32, tag="dp", bufs=2)
                        dps[k] = dp
                        for lane in range(2):
                            j = 2 * k + lane - t["NSe"]  # perc-block index
                            if 2 * k + lane >= t["NP"]:
                                continue
                            nc.tensor.matmul(out=dp[:, lane, :],
                                             lhsT=t["fc"](j),
                                             rhs=t["fq"][:], start=True,
                                             stop=True)
                # batched agg flush (amortizes PE dtype-mode switches);
                # at the drain, p-aggs go first so the big accp output
                # copy/DMA overlaps the remaining s-aggs
                if gi % 2 == 0 or gi >= len(sched):
                    rest = []
                    due = []
                    for job in pend:
                        dly = 5 if job[4] == "a" else 4
                        if job[0] <= gi - dly or gi >= len(sched):
                            due.append(job)
                        else:
                            rest.append(job)
                    if gi >= len(sched):
                        due.sort(key=lambda j: j[2] != "p")
                    for job in due:
                        flush(job)
                    pend = rest
                if grp:
                    t = tiles[s]
                    NS2 = t["NSe"] // 2
                    for k in grp:
                        pt = wpool.tile([128, 2, C], f8, tag="pm", bufs=10)
                        ppair[s][k] = pt
                        odd_last = (2 * k + 1 == t["NPe"]
                                    and t["NPe"] != t["NP"])
                        if odd_last:
                            nc.gpsimd.memset(pt[:, 1, :], 0.0)
                        if k < NS2:
                            # exact sep path: squares + add + dual masks
                            sd = wpool.tile([128, 2, C], f32, tag="sd",
                                            bufs=2)
                            for lane in range(2):
                                b = 2 * k + lane
                                dx2 = wpool.tile([128, C], f32, tag="dx2",
                                                 bufs=2)
                                nc.scalar.activation(
                                    out=dx2[:], in_=t["qx"], func=AF.Square,
                                    bias=t["jbs"][:, 2 * b:2 * b + 1],
                                    scale=1.0)
                                dy2 = wpool.tile([128, C], f32, tag="dy2",
                                                 bufs=2)
                                nc.scalar.activation(
                                    out=dy2[:], in_=t["qy"], func=AF.Square,
                                    bias=t["jbs"][:, 2 * b + 1:2 * b + 2],
                                    scale=1.0)
                                nc.gpsimd.tensor_tensor(
                                    out=sd[:, lane, :], in0=dx2[:],
                                    in1=dy2[:], op=ALU.add)
                            st = wpool.tile([128, 2, C], f8, tag="sm",
                                            bufs=4)
                            spair[s][k] = st
                            emit_mask(eng[("p", 2 * k)], pt[:, :, :],
                                      sd[:, :, :], t["thr"], PERC2)
                            emit_mask(eng[("s", 2 * k)], st[:, :, :],
                                      sd[:, :, :], t["thrs"], SEP2)
                            pend.append((gi, s, "p", k,
                                         eng[("p", 2 * k)]))
                            pend.append((gi, s, "s", k,
                                         eng[("s", 2 * k)]))
                        else:
                            dp = dps[k]
                            e0 = eng[("p", 2 * k)]
                            e1 = eng.get(("p", 2 * k + 1), e0)
                            if odd_last:
                                emit_mask(e0, pt[:, 0, :],
                                          dp[:, 0, :], t["thr"], PERC2)
                            elif e0 != e1:
                                # head pairs: per-lane masks in parallel
                                for lane, e in ((0, e0), (1, e1)):
                                    emit_mask(e, pt[:, lane, :],
                                              dp[:, lane, :],
                                              t["thr"], PERC2)
                            else:
                                emit_mask(e0, pt[:, :, :],
                                          dp[:, :, :], t["thr"], PERC2)
                            pend.append((gi, s, "p", k, e0))
    nc.finalize()
    return nc


def _get_nc(cfg):
    if cfg not in _CACHE:
        _CACHE[cfg] = _build(cfg)
    return _CACHE[cfg]


def _fp8_splits(v64, k):
    """value ~= sum_i stored_i / 16^i, each stored_i an e4m3 array."""
    outs = []
    rem = np.array(v64, np.float64, copy=True)
    for i in range(k):
        s = float(16.0 ** i)
        q = (rem * s).astype(E4M3)
        outs.append(q)
        rem -= q.astype(np.float64) / s
    return outs


def _bsplit2(v64):
    """v -> two bf16 parts (residual ~ 2^-18 |v|)."""
    a1 = v64.astype(BF16)
    a2 = (v64 - a1.astype(np.float64)).astype(BF16)
    return a1, a2


def _features_q(ux, uy):
    """query features [10, n] bf16 from centered coords (f64).

    d2 = qn + cn - 2 u.v; row pairing with _features_c:
    (1,n1v) (1,n2v) (n1u,1) (n2u,1) (a1x,-2b1x) (a1x,-2b2x) (a2x,-2b1x)
    + y rows. |err| <~ 1e-5 (perception threshold only).
    """
    a1x, a2x = _bsplit2(ux)
    a1y, a2y = _bsplit2(uy)
    n1, n2 = _bsplit2(ux * ux + uy * uy)
    one = np.ones_like(a1x)
    return np.stack([one, one, n1, n2,
                     a1x, a1x, a2x, a1y, a1y, a2y]).astype(BF16)


def _features_c(vx, vy, pad):
    """candidate features [10, n] bf16; pad entries get d2 = 64."""
    b1x, b2x = _bsplit2(vx)
    b1y, b2y = _bsplit2(vy)
    n1, n2 = _bsplit2(vx * vx + vy * vy)
    n1 = np.where(pad, BF16(64.0), n1)
    z = np.zeros_like(b1x)
    for a in (b1x, b2x, b1y, b2y, n2):
        np.copyto(a, np.where(pad, z, a))
    one = np.where(pad, z, np.ones_like(b1x))
    return np.stack([n1, n2, one, one,
                     -2 * b1x, -2 * b2x, -2 * b1x,
                     -2 * b1y, -2 * b2y, -2 * b1y]).astype(BF16)


def _pack_block(vals_splits, halve):
    """vals_splits: list of (v64[m], nsplit). Returns (w8 [m,F], eff [m,F])."""
    m = len(vals_splits[0][0])
    F = sum(ns for _, ns in vals_splits)
    w8 = np.zeros((m, F), E4M3)
    mul = 2.0 if halve else 1.0
    h = 0.5 if halve else 1.0
    f = 0
    for v, ns in vals_splits:
        for q in _fp8_splits(v * h, ns):
            w8[:, f] = q
            f += 1
    eff = w8.astype(np.float64) * mul
    return w8, eff


def _prepare(pos, vel):
    n = pos.shape[0]
    assert n == N, f"expected {N} boids, got {n}"

    # --- 16 quantile patches: 4 x-strips x 4 y-quantiles of C queries ---
    xorder = np.argsort(pos[:, 0], kind="stable")
    psel = []
    for s in range(4):
        strip = xorder[(n // 4) * s:(n // 4) * (s + 1)]
        yord = np.argsort(pos[strip, 1], kind="stable")
        for tq in range(4):
            psel.append(strip[yord[C * tq:C * (tq + 1)]])

    p64x = pos[:, 0].astype(np.float64)
    p64y = pos[:, 1].astype(np.float64)
    kx = np.round(p64x * SCL).astype(np.int64)
    ky = np.round(p64y * SCL).astype(np.int64)
    lattice = bool(
        np.all(kx.astype(np.float64) == p64x * SCL)
        and np.all(ky.astype(np.float64) == p64y * SCL)
        and kx.min() >= 0 and kx.max() < SCL
        and ky.min() >= 0 and ky.max() < SCL
    )
    vx64 = vel[:, 0].astype(np.float64)
    vy64 = vel[:, 1].astype(np.float64)

    patches = []
    for sel in psel:
        cxm = 0.5 * (p64x[sel].min() + p64x[sel].max())
        cym = 0.5 * (p64y[sel].min() + p64y[sel].max())
        hx = 0.5 * (p64x[sel].max() - p64x[sel].min()) + 2.0 / SCL
        hy = 0.5 * (p64y[sel].max() - p64y[sel].min()) + 2.0 / SCL
        assert hx + 0.2 < 0.49 and hy + 0.2 < 0.49, (hx, hy)
        axk = int(round(cxm * SCL))
        ayk = int(round(cym * SCL))
        if lattice:
            sxk = (kx - axk + (SCL >> 1)) % SCL
            syk = (ky - ayk + (SCL >> 1)) % SCL
            cx = (sxk.astype(np.float64) / SCL).astype(np.float32)
            cy = (syk.astype(np.float64) / SCL).astype(np.float32)
        else:  # fallback: tiny (~1e-9) inexactness vs reference wrap
            cx = np.mod(p64x - axk / SCL + 0.5, 1.0).astype(np.float32)
            cy = np.mod(p64y - ayk / SCL + 0.5, 1.0).astype(np.float32)

        c64x = cx.astype(np.float64)
        c64y = cy.astype(np.float64)
        ddx = np.maximum(np.abs(c64x - 0.5) - hx, 0.0)
        ddy = np.maximum(np.abs(c64y - 0.5) - hy, 0.0)
        dd2 = ddx * ddx + ddy * ddy
        is_sep = dd2 <= RCULL_S * RCULL_S
        is_perc = dd2 <= RCULL_P * RCULL_P
        sep_idx = np.nonzero(is_sep)[0]
        po_idx = np.nonzero(is_perc & ~is_sep)[0]
        order = np.concatenate([sep_idx, po_idx])
        npb = (len(order) + 127) // 128
        nsb = (len(sep_idx) + 127) // 128
        patches.append(dict(sel=sel, cx=cx, cy=cy, c64x=c64x, c64y=c64y,
                            order=order, npb=npb, nsb=nsb))

    # --- pair patches into cores: richest with poorest by block count ---
    idx = sorted(range(NPATCH), key=lambda i: -patches[i]["npb"])
    pairs = [(idx[i], idx[NPATCH - 1 - i]) for i in range(NCORES)]
    NP0 = max(patches[a]["npb"] for a, _ in pairs)
    NP1 = max(patches[b]["npb"] for _, b in pairs)
    NS0 = min(max(patches[a]["nsb"] for a, _ in pairs), NP0)
    NS1 = min(max(patches[b]["nsb"] for _, b in pairs), NP1)
    NS0 += NS0 & 1  # sep path handles pairs of blocks
    NS1 += NS1 & 1
    NS0 = min(NS0, NP0)
    NS1 = min(NS1, NP1)
    cfg = ((NP0, NS0), (NP1, NS1))

    in_maps = [dict() for _ in range(NCORES)]
    meta = [[None, None] for _ in range(NCORES)]
    for ci, pair in enumerate(pairs):
        auxs = []
        for s, pi in enumerate(pair):
            NP, NSe = cfg[s]
            NPP = NP - NSe
            NPe = NP + (NP & 1)
            p = patches[pi]
            sel, cx, cy = p["sel"], p["cx"], p["cy"]
            c64x, c64y, order = p["c64x"], p["c64y"], p["order"]
            qx = cx[sel]
            qy = cy[sel]
            fq = _features_q(qx.astype(np.float64) - 0.5,
                             qy.astype(np.float64) - 0.5)

            jobs = _mask_jobs(NP, NSe)
            peng = {k: e for kind, k, e in jobs if kind == "p"}
            seng = {k: e for kind, k, e in jobs if kind == "s"}

            nord = len(order)
            jw = np.zeros((128, 32 * (NPe // 2) + 32 * (NSe // 2)), E4M3)
            jbs = np.full((128, 2 * NSe), -50.0, np.float32)
            fcx = np.zeros(128 * NPP, np.float64)
            fcy = np.zeros(128 * NPP, np.float64)
            fpad = np.ones(128 * NPP, bool)
            corrp = np.zeros(MP, np.float64)
            corrs = np.zeros(MS, np.float64)
            for b in range(NP):
                lo_i = 128 * b
                if lo_i >= nord:
                    break
                jj = order[lo_i:lo_i + 128]
                m = len(jj)
                pxv = c64x[jj] - 0.5
                pyv = c64y[jj] - 0.5
                if b >= NSe:
                    jd = b - NSe
                    fcx[128 * jd:128 * jd + m] = pxv
                    fcy[128 * jd:128 * jd + m] = pyv
                    fpad[128 * jd:128 * jd + m] = False
                else:
                    jbs[:m, 2 * b] = -cx[jj]
                    jbs[:m, 2 * b + 1] = -cy[jj]
                one = np.ones(m, np.float64)
                ep = peng[b]
                w8, eff = _pack_block(
                    [(one, 1), (vx64[jj], 2), (vy64[jj], 2),
                     (pxv, 3), (pyv, 3)], halve=(ep == "a"))
                col = 32 * (b // 2) + 16 * (b & 1)
                jw[:m, col:col + MP] = w8
                corrp += 0.5 * eff.sum(axis=0)
                if b < NSe:
                    es = seng[b]
                    w8s, effs = _pack_block(
                        [(one, 1), (pxv, 4), (pyv, 4)], halve=(es == "a"))
                    col = 32 * (NPe // 2) + 32 * (b // 2) + 16 * (b & 1)
                    jw[:m, col:col + MS] = w8s
                    if es != "g":  # Pool masks are plain 0/1: no offset
                        corrs += 0.5 * effs.sum(axis=0)
            fc = _features_c(fcx, fcy, fpad)

            im = in_maps[ci]
            im[f"ffc{s}"] = np.concatenate([fq, fc], axis=1)
            im[f"jw{s}"] = jw
            qt = np.empty((128, 2 * C), np.float32)
            qt[:, 0:C] = qx[None, :]
            qt[:, C:2 * C] = qy[None, :]
            im[f"qxy{s}"] = qt
            av = np.empty((128, 2 + 2 * NSe), np.float32)
            av[:, 0] = PERC2
            av[:, 1] = SEP2
            av[:, 2:] = jbs
            auxs.append(av)
            meta[ci][s] = dict(
                sel=sel,
                qxc=qx.astype(np.float64) - 0.5,
                qyc=qy.astype(np.float64) - 0.5,
                corrp=corrp,
                corrs=corrs,
            )
        in_maps[ci]["aux"] = np.concatenate(auxs, axis=1)
    return in_maps, meta, cfg


def _recomb(rows, k):
    out = rows[0].copy()
    for i in range(1, k):
        out += rows[i] / (16.0 ** i)
    return out


def kernel(position, velocity, noise, separation_weight, alignment_weight,
           cohesion_weight, noise_scale):
    pos = np.asarray(position, dtype=np.float32)
    vel = np.asarray(velocity, dtype=np.float32)
    noi = np.asarray(noise, dtype=np.float32)
    ws = float(separation_weight)
    wa = float(alignment_weight)
    wc = float(cohesion_weight)
    nsc = float(noise_scale)

    in_maps, meta, cfg = _prepare(pos, vel)
    vx64 = vel[:, 0].astype(np.float64)
    vy64 = vel[:, 1].astype(np.float64)

    nc = _get_nc(cfg)
    res = run_bass_kernel_spmd(nc, in_maps, list(range(NCORES))).results

    out = np.zeros((N, 2), np.float32)
    for ci in range(NCORES):
        for s in range(2):
            md = meta[ci][s]
            sel = md["sel"]
            P = res[ci]["outp"][:, C * s:C * (s + 1)].astype(np.float64)
            S = res[ci]["outs"][:, C * s:C * (s + 1)].astype(np.float64)
            P = P + md["corrp"][:, None]
            S = S + md["corrs"][:, None]
            cnt_all = P[0]
            svx = _recomb(P[1:3], 2)
            svy = _recomb(P[3:5], 2)
            spx = _recomb(P[5:8], 3)
            spy = _recomb(P[8:11], 3)
            scn = S[0]
            ssx = _recomb(S[1:5], 4)
            ssy = _recomb(S[5:9], 4)
            qxc, qyc = md["qxc"], md["qyc"]

            cnt = cnt_all - 1.0
            vax = (svx - vx64[sel]) / cnt
            vay = (svy - vy64[sel]) / cnt
            dvx = vax - vx64[sel]
            dvy = vay - vy64[sel]
            pax = (spx - qxc * cnt_all) / cnt
            pay = (spy - qyc * cnt_all) / cnt
            sepx = -(ssx - qxc * scn)
            sepy = -(ssy - qyc * scn)
            # boids with zero sep neighbors (scn counts self, exactly):
            # reference yields a zero steer; quantization residue would
            # otherwise normalize into a random unit vector
            nosep = (scn - 1.0) < 0.5
            sepx = np.where(nosep, 0.0, sepx)
            sepy = np.where(nosep, 0.0, sepy)

            n1 = np.maximum(np.sqrt(sepx * sepx + sepy * sepy), EPS)
            n2 = np.maximum(np.sqrt(dvx * dvx + dvy * dvy), EPS)
            n3 = np.maximum(np.sqrt(pax * pax + pay * pay), EPS)

            ax = ws * sepx / n1 + wa * dvx / n2 + wc * pax / n3
            ay = ws * sepy / n1 + wa * dvy / n2 + wc * pay / n3
            ax = ax + nsc * noi[sel, 0].astype(np.float64)
            ay = ay + nsc * noi[sel, 1].astype(np.float64)
            nn = np.sqrt(ax * ax + ay * ay)
            f = np.where(nn > 1.0, 1.0 / np.maximum(nn, EPS), 1.0)
            out[sel, 0] = (ax * f).astype(np.float32)
            out[sel, 1] = (ay * f).astype(np.float32)
    return out


def run_with_trace(np_inputs):
    """Debug helper for test.py: run the device program with trace=True and
    return (exec_time_ns, profile_json_path_or_None)."""
    pos = np.asarray(np_inputs["position"], dtype=np.float32)
    vel = np.asarray(np_inputs["velocity"], dtype=np.float32)
    in_maps, _, cfg = _prepare(pos, vel)
    nc = _get_nc(cfg)
    r = run_bass_kernel_spmd(nc, in_maps, list(range(NCORES)), trace=True)
    return getattr(r, "exec_time_ns", None), getattr(r, "profile_json", None)
